# revision 37
# baseline (speedup 1.0000x reference)
"""GNN NodeModel kernel for 8 Trainium2 NeuronCores.

Strategy: shard edges by DESTINATION node block (512 nodes), so scatter_mean
is fully core-local (no collectives).

Key algebraic fusion: MLP1's second layer is linear and commutes with
scatter_mean, and its output only feeds MLP2's first layer (also linear
pre-ReLU).  So
    scatter_mean(relu(a@W1a+b1a) @ W1b + b1b) @ W2a_mid
  = scatter_mean(relu(a@W1a+b1a)) @ (W1b @ W2a_mid) + [cnt>0] * (b1b @ W2a_mid)
with W1b @ W2a_mid precomputed on the host.  The entire per-edge second MLP1
layer disappears; the scatter operates on h1 directly.

Per core:
  - edge-parallel MLP1 layer 1 with edges as the PSUM partition dim
    (stationary = input chunks, moving = W1a); b1a folded in via a ones-row
  - scatter-add via one-hot S-matrix matmuls into per-node-block PSUM
    accumulators; the scatter_mean 1/count is folded INTO S entries
  - node-parallel MLP2 on aggregated h1, with the fused (W1b@W2a_mid) weight
    and a [cnt>0] mask carried as a 26th per-node feature
All 8 cores run one shared SPMD program; per-node-block edge counts are made
structurally identical across cores by grouping blocks with matching padded
window sizes and padding each rank to the max across cores.
"""

import os
import sys

sys.path.insert(0, "/opt/trn_rl_repo")

import numpy as np
import ml_dtypes

import concourse.bass as bass
import concourse.mybir as mybir
import concourse.tile as tile
from concourse import bacc
from concourse.bass_utils import run_bass_kernel_spmd

P = 128          # partitions
H = 512          # hidden width
NBN = 512        # nodes per node-block (MLP2 unit)
SW = 256         # nodes per scatter window (2 windows per node-block)
EB = 512         # edges per compute block
NCORES = 8

F32 = mybir.dt.float32
BF16 = mybir.dt.bfloat16
I32 = mybir.dt.int32
NP_BF16 = ml_dtypes.bfloat16

LAST_RUN_INFO = {}


def _build_structure(row, n_nodes):
    """Partition node blocks across cores; compute shared slot structure.

    Scatter operates on 256-node windows (2 per 512-node MLP2 block), so the
    per-rank capacities C are per-window [2 * nbk], each a multiple of 128.
    Blocks are grouped into ranks by matching (wpad0, wpad1) pairs so the
    cross-core rank-wise max adds minimal padding.
    """
    n_blocks_g = -(-n_nodes // NBN)
    wcnt = np.bincount(row // SW, minlength=2 * n_blocks_g).astype(np.int64)
    wpad = np.maximum(P, ((wcnt + P - 1) // P) * P)
    bp = wpad.reshape(-1, 2)

    # group blocks with identical padded window pairs; leftovers by sorted deal
    buckets = {}
    for g in range(n_blocks_g):
        buckets.setdefault((int(bp[g][0]), int(bp[g][1])), []).append(g)
    groups = []
    leftovers = []
    for key in sorted(buckets, reverse=True):
        v = buckets[key]
        while len(v) >= NCORES:
            groups.append(v[:NCORES])
            v = v[NCORES:]
        leftovers.extend(v)
    leftovers.sort(key=lambda g: (-bp[g][0], -bp[g][1]))
    groups.extend(
        leftovers[i:i + NCORES] for i in range(0, len(leftovers), NCORES)
    )

    nbk = len(groups)
    core_blocks = [[] for _ in range(NCORES)]
    C = np.zeros(2 * nbk, dtype=np.int64)
    for j, grp in enumerate(groups):
        C[2 * j] = max(int(bp[g][0]) for g in grp)
        C[2 * j + 1] = max(int(bp[g][1]) for g in grp)
        for k, g in enumerate(grp):
            core_blocks[k].append(int(g))
    et = int(C.sum())
    rem = (-et) % EB
    C[-1] += rem
    et += rem
    return core_blocks, nbk, C, et, wcnt


def _build_program(nbk, C, et, trace_sim=False, reps=1):
    """Trace the shared SPMD Bass program for the given slot structure."""
    sub = et // P
    ebk = et // EB
    npad = nbk * NBN

    # sub-tile t -> (window slot jw, first?, last?)
    sub_first = {}
    sub_last = {}
    sub_win = np.empty(sub, dtype=np.int64)
    t = 0
    for jw in range(2 * nbk):
        ns = int(C[jw]) // P
        for s in range(ns):
            sub_win[t] = jw
            if s == 0:
                sub_first[t] = True
            if s == ns - 1:
                sub_last[t] = True
            t += 1
    assert t == sub

    nc = bacc.Bacc("TRN2", target_bir_lowering=False, debug=False)
    A0 = nc.declare_dram_parameter("a0", [P, 4, et], BF16, isOutput=False)
    A1 = nc.declare_dram_parameter("a1", [10, et], BF16, isOutput=False)
    DLOC = nc.declare_dram_parameter("dloc", [P, sub], F32, isOutput=False)
    INVE = nc.declare_dram_parameter("inve", [P, sub], F32, isOutput=False)
    XU = nc.declare_dram_parameter("xu", [26, npad], BF16, isOutput=False)
    W1A = nc.declare_dram_parameter("w1a", [P, 4, H], BF16, isOutput=False)
    W1AX = nc.declare_dram_parameter("w1ax", [10, H], BF16, isOutput=False)
    WF = nc.declare_dram_parameter("wf", [P, 4, H], BF16, isOutput=False)
    W2AX = nc.declare_dram_parameter("w2ax", [26, H], BF16, isOutput=False)
    W2B = nc.declare_dram_parameter("w2b", [P, 4], BF16, isOutput=False)
    B2A = nc.declare_dram_parameter("b2a", [P, 4], F32, isOutput=False)
    OUT = nc.declare_dram_parameter("out", [1, npad], F32, isOutput=True)

    with tile.TileContext(nc, trace_sim=trace_sim) as tc:
        with (
            tc.tile_pool(name="wpool", bufs=1) as wpool,
            tc.tile_pool(name="apool", bufs=6) as apool,
            tc.tile_pool(name="hpool", bufs=11) as hpool,
            tc.tile_pool(name="spool", bufs=11) as spool,
            tc.tile_pool(name="ztpool", bufs=3) as ztpool,
            tc.tile_pool(name="ttpool", bufs=3) as ttpool,
            tc.tile_pool(name="mmps", bufs=5, space="PSUM") as mmps,
            tc.tile_pool(name="aggps", bufs=1, space="PSUM") as aggps,
            tc.tile_pool(name="outps", bufs=1, space="PSUM") as outps,
        ):
            # ---- constants / weights ----
            w1a = wpool.tile([P, 4, H], BF16)
            nc.sync.dma_start(w1a[:], W1A[:])
            w1ax = wpool.tile([10, H], BF16)
            nc.sync.dma_start(w1ax[:], W1AX[:])
            wf = wpool.tile([P, 4, H], BF16)
            nc.sync.dma_start(wf[:], WF[:])
            w2ax = wpool.tile([26, H], BF16)
            nc.sync.dma_start(w2ax[:], W2AX[:])
            w2b = wpool.tile([P, 4], BF16)
            nc.sync.dma_start(w2b[:], W2B[:])
            b2a = wpool.tile([P, 4], F32)
            nc.sync.dma_start(b2a[:], B2A[:])
            dloc = wpool.tile([P, sub], F32)
            nc.sync.dma_start(dloc[:], DLOC[:])
            inve = wpool.tile([P, sub], F32)
            nc.sync.dma_start(inve[:], INVE[:])

            iota_i = wpool.tile([P, SW], I32)
            nc.gpsimd.iota(iota_i[:], pattern=[[1, SW]], base=0, channel_multiplier=0)
            iota_f = wpool.tile([P, SW], BF16)
            nc.vector.tensor_copy(iota_f[:], iota_i[:])

            out_row = wpool.tile([1, npad], F32)

            agg_bank = [None, None]  # psum banks; each holds 2 h-chunks
            agg = [None] * 4         # per-h-chunk views into the banks
            cur_zt = [None]   # zT staging tile of the in-flight node block
            xu_tiles = {}     # node block j -> prefetched xu tile

            def mlp2_head(j, zt):
                xu = xu_tiles.pop(j)
                tts = []
                for m in range(4):
                    pst = mmps.tile([P, H], F32, tag="mm")
                    for k in range(4):
                        nc.tensor.matmul(
                            pst[:], wf[:, k, m * P:(m + 1) * P], zt[:, k, :],
                            start=(k == 0), stop=False,
                        )
                    nc.tensor.matmul(
                        pst[:], w2ax[:, m * P:(m + 1) * P], xu[:],
                        start=False, stop=True,
                    )
                    tt = ttpool.tile([P, NBN], BF16, name=f"tt{m}")
                    nc.scalar.activation(
                        tt[:], pst[:], mybir.ActivationFunctionType.Relu,
                        bias=b2a[:, m:m + 1],
                    )
                    tts.append(tt)
                return tts

            def mlp2_tail(j, tts):
                ops = outps.tile([1, NBN], F32, tag="outps")
                for k in range(4):
                    nc.tensor.matmul(
                        ops[:], w2b[:, k:k + 1], tts[k][:],
                        start=(k == 0), stop=(k == 3),
                    )
                nc.vector.tensor_copy(out_row[0:1, j * NBN:(j + 1) * NBN], ops[:])

            def emit_scatter(t, h1t, s_t):
                # scatter: aggT[m] += h1[:, m-chunk].T @ S (256-node window)
                # S entries carry 1/count so eviction is a plain copy
                jw = int(sub_win[t])
                j, half = jw // 2, jw % 2
                first = sub_first.get(t, False)
                last = sub_last.get(t, False)
                if first:
                    for q in range(2):
                        agg_bank[q] = aggps.tile(
                            [P, 2, SW], F32, tag=f"agg{q}", name=f"agg{q}")
                    for m in range(4):
                        agg[m] = agg_bank[m // 2][:, m % 2, :]
                for m in range(4):
                    # start clears has_written for the WHOLE bank, so only the
                    # bank's first matmul (even m, first sub-tile) may set it;
                    # the odd sibling then overwrites-where-unset on the
                    # freshly cleared bank.
                    nc.tensor.matmul(
                        agg[m], h1t[:, m * P:(m + 1) * P],
                        s_t[:], start=(first and m % 2 == 0),
                        stop=(last and m % 2 == 1), skip_group_check=True,
                    )
                if last:
                    # evict aggT -> zT (mean division already folded into S);
                    # one whole-bank copy per bank so the DVE read depends on
                    # every PE write to that bank
                    if half == 0:
                        cur_zt[0] = ztpool.tile([P, 4, NBN], BF16, name="zt")
                    zt = cur_zt[0]
                    for q in range(2):
                        nc.vector.tensor_copy(
                            zt[:, 2 * q:2 * q + 2, half * SW:(half + 1) * SW],
                            agg_bank[q][:],
                        )
                    if half == 1:
                        return (j, zt)
                return None

            # ---- main loop over edge blocks, software-pipelined so the PE
            # never waits on scalar/vector evictions (keeps the p-state high):
            # scatter for sub-tile t issues after L1 of t+SKEW_SC; MLP2 is
            # similarly delayed, with its xu operand prefetched at window start
            SKEW_SC = 8
            SKEW_M2 = 4
            SKEW_MT = 3
            for _rep in range(reps):
              pend_sc = []   # (t, h1t, s_t)
              pend_m2 = []   # (t_queued, j, zt)
              pend_mt = []   # (t_queued, j, tts)
              for b in range(ebk):
                a0 = apool.tile([P, 4, EB], BF16, name="a0")
                nc.sync.dma_start(a0[:], A0[:, :, b * EB:(b + 1) * EB])
                a1 = apool.tile([10, EB], BF16, name="a1")
                nc.sync.dma_start(a1[:], A1[:, b * EB:(b + 1) * EB])

                for es in range(4):
                    e0 = es * P
                    t = b * 4 + es
                    jw = int(sub_win[t])
                    if sub_first.get(t, False) and jw % 2 == 0:
                        j = jw // 2
                        xu = apool.tile([26, NBN], BF16, name="xu")
                        nc.sync.dma_start(xu[:], XU[:, j * NBN:(j + 1) * NBN])
                        xu_tiles[j] = xu

                    # MLP1 layer 1, edge-major: h1[e, :] = relu(a_e @ W1a)
                    # (b1a rides the ones-row of a1 / w1ax row 9)
                    ps = mmps.tile([P, H], F32, tag="mm")
                    for k in range(4):
                        nc.tensor.matmul(
                            ps[:], a0[:, k, e0:e0 + P], w1a[:, k, :],
                            start=(k == 0), stop=False,
                        )
                    nc.tensor.matmul(
                        ps[:], a1[:, e0:e0 + P], w1ax[:],
                        start=False, stop=True,
                    )
                    h1t = hpool.tile([P, H], BF16, name="h1")
                    nc.scalar.activation(
                        h1t[:], ps[:], mybir.ActivationFunctionType.Relu,
                    )
                    s_t = spool.tile([P, SW], BF16, name="s")
                    nc.vector.tensor_scalar(
                        out=s_t[:], in0=iota_f[:],
                        scalar1=dloc[:, t:t + 1], scalar2=inve[:, t:t + 1],
                        op0=mybir.AluOpType.is_equal, op1=mybir.AluOpType.mult,
                    )

                    pend_sc.append((t, h1t, s_t))
                    while len(pend_sc) > SKEW_SC:
                        m2 = emit_scatter(*pend_sc.pop(0))
                        if m2 is not None:
                            pend_m2.append((t, *m2))
                    while pend_m2 and t - pend_m2[0][0] >= SKEW_M2:
                        _, j, zt = pend_m2.pop(0)
                        pend_mt.append((t, j, mlp2_head(j, zt)))
                    while pend_mt and t - pend_mt[0][0] >= SKEW_MT:
                        _, j, tts = pend_mt.pop(0)
                        mlp2_tail(j, tts)

              for item in pend_sc:
                  m2 = emit_scatter(*item)
                  if m2 is not None:
                      pend_m2.append((0, *m2))
              for _, j, zt in pend_m2:
                  pend_mt.append((0, j, mlp2_head(j, zt)))
              for _, j, tts in pend_mt:
                  mlp2_tail(j, tts)

            nc.sync.dma_start(OUT[:], out_row[:])

    if not trace_sim:
        nc.compile()
    return nc


def kernel(**inputs):
    x = np.ascontiguousarray(np.asarray(inputs["x"], dtype=np.float32))
    edge_index = np.asarray(inputs["edge_index"], dtype=np.int64)
    edge_attr = np.ascontiguousarray(np.asarray(inputs["edge_attr"], dtype=np.float32))
    u = np.asarray(inputs["u"], dtype=np.float32)
    batch = np.asarray(inputs["batch"], dtype=np.int64)
    W1a = np.asarray(inputs["W1a"], dtype=np.float32)
    b1a = np.asarray(inputs["b1a"], dtype=np.float32)
    W1b = np.asarray(inputs["W1b"], dtype=np.float32)
    b1b = np.asarray(inputs["b1b"], dtype=np.float32)
    W2a = np.asarray(inputs["W2a"], dtype=np.float32)
    b2a = np.asarray(inputs["b2a"], dtype=np.float32)
    W2b = np.asarray(inputs["W2b"], dtype=np.float32)
    b2b = np.asarray(inputs["b2b"], dtype=np.float32)

    n_nodes = x.shape[0]
    row, col = edge_index[0], edge_index[1]

    cnt = np.bincount(row, minlength=n_nodes)
    inv = (1.0 / np.maximum(cnt, 1)).astype(np.float32)
    mask = (cnt > 0).astype(np.float32)

    core_blocks, nbk, C, et, wcnt = _build_structure(row, n_nodes)
    sub = et // P
    npad = nbk * NBN
    Cstart = np.concatenate([[0], np.cumsum(C)])

    nc = _build_program(nbk, C, et)

    # ---- per-core shards ----
    order = np.argsort(row, kind="stable")
    wstart = np.concatenate([[0], np.cumsum(wcnt)])

    # weights (shared by all cores)
    W2a_mid = W2a[9:521]                      # [512, 512] agg rows
    WF_f = W1b @ W2a_mid                      # host-fused [512, 512]
    W1a_m = np.ascontiguousarray(
        W1a[9:521].reshape(4, P, H).transpose(1, 0, 2).astype(NP_BF16))
    W1a_x = np.ascontiguousarray(
        np.vstack([W1a[0:9], b1a[None, :]]).astype(NP_BF16))
    WF_r = np.ascontiguousarray(
        WF_f.reshape(4, P, H).transpose(1, 0, 2).astype(NP_BF16))
    W2a_x = np.ascontiguousarray(
        np.vstack([W2a[0:9], W2a[521:537], (b1b @ W2a_mid)[None, :]]).astype(NP_BF16))
    W2b_r = np.ascontiguousarray(W2b[:, 0].reshape(4, P).T.astype(NP_BF16))
    b2a_r = np.ascontiguousarray(b2a.reshape(4, P).T)

    xT = x.T  # [9, N]
    uT_b = u[batch].T  # [16, N]

    in_maps = []
    core_slot_blocks = []
    for k in range(NCORES):
        blocks = core_blocks[k] + [-1] * (nbk - len(core_blocks[k]))
        core_slot_blocks.append(blocks)
        eidx = np.full(et, -1, dtype=np.int64)
        for j, g in enumerate(blocks):
            if g >= 0:
                for h in range(2):
                    gw = 2 * g + h
                    ne = int(wcnt[gw])
                    s0 = Cstart[2 * j + h]
                    eidx[s0:s0 + ne] = order[wstart[gw]:wstart[gw] + ne]
        valid = eidx >= 0
        e_safe = np.where(valid, eidx, 0)

        ea = edge_attr[e_safe]  # [et, 512]
        A0 = np.ascontiguousarray(
            ea.T.reshape(4, P, et).transpose(1, 0, 2).astype(NP_BF16))
        A1 = np.ascontiguousarray(
            np.vstack([x[col[e_safe]].T, np.ones((1, et), np.float32)]
                      ).astype(NP_BF16))  # [10, et]

        dest = row[e_safe]
        # dest-local index within the slot's scatter window
        win_of_slot = np.repeat(np.arange(2 * nbk), C)
        gwin = np.array(
            [2 * blocks[jw // 2] + jw % 2 if blocks[jw // 2] >= 0 else -1
             for jw in win_of_slot], dtype=np.int64)
        dl_val = (dest - gwin * SW).astype(np.float32)
        dl = np.where(valid, dl_val, -1.0).astype(np.float32)
        dloc_a = np.ascontiguousarray(dl.reshape(sub, P).T)
        ive = np.where(valid, inv[dest], 1.0).astype(np.float32)
        inve_a = np.ascontiguousarray(ive.reshape(sub, P).T)

        xu_a = np.zeros((26, npad), dtype=np.float32)
        for j, g in enumerate(blocks):
            if g < 0:
                continue
            lo = g * NBN
            hi = min(lo + NBN, n_nodes)
            w = hi - lo
            xu_a[0:9, j * NBN:j * NBN + w] = xT[:, lo:hi]
            xu_a[9:25, j * NBN:j * NBN + w] = uT_b[:, lo:hi]
            xu_a[25, j * NBN:j * NBN + w] = mask[lo:hi]

        in_maps.append({
            "a0": A0, "a1": A1, "dloc": dloc_a, "inve": inve_a,
            "xu": np.ascontiguousarray(xu_a.astype(NP_BF16)),
            "w1a": W1a_m, "w1ax": W1a_x, "wf": WF_r,
            "w2ax": W2a_x, "w2b": W2b_r, "b2a": b2a_r,
        })

    res = run_bass_kernel_spmd(nc, in_maps, core_ids=list(range(NCORES)), trace=False)
    LAST_RUN_INFO.clear()
    LAST_RUN_INFO.update({
        "exec_time_ns": res.exec_time_ns,
        "nc": nc,
        "in_maps": in_maps,
        "structure": (nbk, C, et),
    })

    out_full = np.zeros(n_nodes, dtype=np.float32)
    for k in range(NCORES):
        o = res.results[k]["out"][0]
        for j, g in enumerate(core_slot_blocks[k]):
            if g < 0:
                continue
            lo = g * NBN
            hi = min(lo + NBN, n_nodes)
            out_full[lo:hi] = o[j * NBN:j * NBN + (hi - lo)]

    result = out_full[:, None] + b2b[None, :] if b2b.ndim == 1 else out_full[:, None] + b2b
    return result.astype(np.float32)


def _bench_build(nc, in_maps, reps):
    """Build a jitted SPMD executable running the NEFF `reps` times back-to-back."""
    import jax
    import jax.numpy as jnp
    from jax.sharding import Mesh, PartitionSpec
    from jax.experimental.shard_map import shard_map

    from concourse import bass2jax
    from concourse import mybir as _mybir

    bass2jax.install_neuronx_cc_hook()
    partition_name = nc.partition_id_tensor.name if nc.partition_id_tensor else None

    in_names, out_names, out_avals, zero_outs = [], [], [], []
    for alloc in nc.m.functions[0].allocations:
        if not isinstance(alloc, _mybir.MemoryLocationSet):
            continue
        name = alloc.memorylocations[0].name
        if alloc.kind == "ExternalInput":
            if name != partition_name:
                in_names.append(name)
        elif alloc.kind == "ExternalOutput":
            shape = tuple(alloc.tensor_shape)
            dtype = _mybir.dt.np(alloc.dtype)
            out_names.append(name)
            out_avals.append(jax.core.ShapedArray(shape, dtype))
            zero_outs.append(np.zeros(shape, dtype))
    n_params = len(in_names)
    all_in_names = in_names + out_names
    if partition_name is not None:
        all_in_names.append(partition_name)

    bind_kw = dict(
        out_avals=tuple(out_avals),
        in_names=tuple(all_in_names),
        out_names=tuple(out_names),
        lowering_input_output_aliases=(),
        sim_require_finite=True,
        sim_require_nnan=True,
        nc=nc,
    )

    assert reps == 1

    def _body(*args):
        operands = list(args)
        if partition_name is not None:
            operands.append(bass2jax.partition_id_tensor())
        outs = bass2jax._bass_exec_p.bind(*operands, **bind_kw)
        return tuple(outs)

    n_cores = len(in_maps)
    devices = jax.devices()[:n_cores]
    mesh = Mesh(np.asarray(devices), ("core",))
    in_specs = (PartitionSpec("core"),) * (n_params + len(out_names))
    out_specs = (PartitionSpec("core"),) * len(out_names)
    fn = jax.jit(
        shard_map(_body, mesh=mesh, in_specs=in_specs, out_specs=out_specs,
                  check_rep=False),
        keep_unused=True,
    )
    concat_in = [
        np.concatenate([np.asarray(in_maps[c][nm]) for c in range(n_cores)], axis=0)
        for nm in in_names
    ] + [np.concatenate([z] * n_cores, axis=0) for z in zero_outs]
    sharding = jax.sharding.NamedSharding(mesh, PartitionSpec("core"))
    args = [jax.device_put(a, sharding) for a in concat_in]
    return fn, args


def _pipe_time(fn, args, n_pipe, iters):
    import time

    fn(*args)[0].block_until_ready()  # warm
    best = float("inf")
    for _ in range(iters):
        t0 = time.perf_counter()
        outs = [fn(*args) for _ in range(n_pipe)]
        outs[-1][0].block_until_ready()
        best = min(best, (time.perf_counter() - t0) / n_pipe)
    return best


def bench(r_lo=5, r_hi=10, n_pipe=64, iters=3):
    """Per-NEFF-body exec time: marginal cost between r_hi-x and r_lo-x
    replicated bodies, both deep enough that device execution (not dispatch
    RPC) is the pipeline bottleneck."""
    in_maps = LAST_RUN_INFO["in_maps"]
    st = LAST_RUN_INFO["structure"]

    times = {}
    for r in (r_lo, r_hi):
        ncR = _build_program(*st, reps=r)
        fnR, argsR = _bench_build(ncR, in_maps, 1)
        times[r] = _pipe_time(fnR, argsR, n_pipe, iters)
    exec_ns = (times[r_hi] - times[r_lo]) / (r_hi - r_lo) * 1e9
    LAST_RUN_INFO["exec_time_ns"] = exec_ns
    LAST_RUN_INFO["bench_detail"] = {f"t{r}_ms": f"{t * 1e3:.2f}" for r, t in times.items()}
    return exec_ns
